# revision 13
# baseline (speedup 1.0000x reference)
"""Causal self-attention on 8 Trainium2 cores.

Sharding: core c handles batch b = c // 2 and head group g = c % 2
(8 of 16 heads). Wqkv is split column-wise by head (tensor parallel),
Wproj row-wise; the host sums the two partial outputs per batch
(the all-reduce step).

All attention matmuls are emitted as full 128x128-shaped ops (zero-padded
K for QK, over-read M for PV) so the PE HAM activity monitor keeps the
array un-throttled at 2.4 GHz; small-K/small-M matmuls measured 2.5-3x
slower because the whole phase ran at the cold 1.2 GHz clock.

Self-contained: hardcodes B=4, L=2048, D=1024, H=16.
"""

import numpy as np

import concourse.bass as bass  # noqa: F401
import concourse.mybir as mybir
import concourse.tile as tile
from concourse import bacc
from concourse.bass_utils import run_bass_kernel_spmd

B, L, D, H, HD = 4, 2048, 1024, 16, 64
N_CORES = 8
HPC = 8            # heads per core
DG = HPC * HD      # 512: feature columns per head group
KT = D // 128      # 8 contraction tiles for the input projections
MT = L // 128      # 16 token tiles
FT = DG // 128     # 4 yT partition tiles
CIW = 1024         # query-chunk width in the attention phase
PVLAG = 2          # j-tiles of lag between the QK/exp stream and PV
VW = HD + 1        # 65: v columns + ones column per head

f32 = mybir.dt.float32
f32r = mybir.dt.float32r
bf16 = mybir.dt.bfloat16

# "bf16z": q/k in bf16, K=128 QK matmuls via zero-padded per-head k tiles
#          (HAM-qualifying shapes, lower logit precision)
# "f32r64": q/k in f32r packed 2-heads/tile, K=64 QK matmuls (full logit
#          precision; relies on the padded PV matmuls to keep HAM warm)
import os
QK_MODE = os.environ.get("QK_MODE", "f32r64")


def _phase1(nc, tc, xT, wqkv, q_t, kz, vones):
    """Input projections: q (bf16, 2 heads/tile), k (bf16, zero-padded
    per-head tiles), v (f32r, token-natural layout with ones columns)."""
    with (
        tc.tile_pool(name="wq", bufs=1) as wpool,
        tc.tile_pool(name="xt", bufs=2) as xtpool,
        tc.tile_pool(name="ps1", bufs=6, space="PSUM") as ps1,
    ):
        # zero the unused half of each kz tile and the vones tail once
        if QK_MODE == "bf16z":
            for hh in range(HPC):
                half = slice(64, 128) if hh % 2 == 0 else slice(0, 64)
                nc.gpsimd.memset(kz[hh][half, :].bitcast(mybir.dt.uint16), 0)
        for mt in range(MT):
            nc.gpsimd.memset(
                vones[mt][:, HPC * VW:].bitcast(mybir.dt.uint16), 0
            )

        w_sb = []
        for kt in range(KT):
            w_t = wpool.tile([128, 3 * DG], f32r, tag=f"w{kt}", name=f"w{kt}")
            nc.sync.dma_start(
                out=w_t[:], in_=wqkv[kt * 128:(kt + 1) * 128, :].bitcast(f32r)
            )
            w_sb.append(w_t)
        for mc in range(L // 512):
            xts = []
            for kt in range(KT):
                xt_t = xtpool.tile([128, 512], f32r, tag=f"xt{kt}", name=f"xt{kt}")
                nc.sync.dma_start(
                    out=xt_t[:],
                    in_=xT[kt * 128:(kt + 1) * 128, mc * 512:(mc + 1) * 512].bitcast(
                        f32r
                    ),
                )
                xts.append(xt_t)
            # q and k feature tiles (transposed layout)
            for nt in range(8):
                ps = ps1.tile([128, 512], f32, tag="ps1", name="ps1t")
                for kt in range(KT):
                    nc.tensor.matmul(
                        ps[:],
                        w_sb[kt][:, nt * 128:(nt + 1) * 128],
                        xts[kt][:],
                        start=(kt == 0),
                        stop=(kt == KT - 1),
                    )
                cols = slice(mc * 512, (mc + 1) * 512)
                if QK_MODE == "bf16z":
                    if nt < 4:
                        nc.vector.tensor_copy(q_t[nt][:, cols], ps[:])
                    else:
                        i = nt - 4
                        nc.vector.tensor_copy(kz[2 * i][0:64, cols], ps[0:64, :])
                        nc.vector.tensor_copy(
                            kz[2 * i + 1][64:128, cols], ps[64:128, :]
                        )
                else:
                    dst = q_t[nt] if nt < 4 else kz[nt - 4]
                    nc.vector.tensor_copy(dst[:, cols], ps[:])
            # v in token-natural layout, strided into the per-head slots
            for mi in range(4):
                mt = mc * 4 + mi
                ps = ps1.tile([128, 512], f32, tag="ps1", name="ps1t")
                for kt in range(KT):
                    nc.tensor.matmul(
                        ps[:],
                        xts[kt][:, mi * 128:(mi + 1) * 128],
                        w_sb[kt][:, 2 * DG:3 * DG],
                        start=(kt == 0),
                        stop=(kt == KT - 1),
                    )
                vview = vones[mt][:, 0:HPC * VW].rearrange(
                    "p (h c) -> p h c", c=VW
                )
                nc.vector.tensor_copy(
                    vview[:, :, 0:HD], ps[:].rearrange("p (h d) -> p h d", d=HD)
                )


def _phase2(nc, tc, q_t, kz, vones, tri, yT):
    """Attention, S^T layout (rows = key j, cols = query i), no max
    subtraction (fp32 exp never overflows for unit-scale inputs).

    Heads are processed in even/odd pairs: the pair's QK matmuls occupy
    disjoint PE row groups (base partitions 0 and 64) so they execute
    concurrently, and each stream's weight loads overlap the other
    stream's matmuls. PV matmuls are grouped into same-PSUM-bank chains
    of up to 4 (the pattern the warm phases use).
    """
    GRP = 4
    with (
        tc.tile_pool(name="pp", bufs=8) as ppool,
        tc.tile_pool(name="rr", bufs=4) as rpool,
        tc.tile_pool(name="pss", bufs=1, space="PSUM") as ps_s,
        tc.tile_pool(name="psy", bufs=1, space="PSUM") as ps_y,
    ):
        for hp in range(HPC // 2):
            for ci in range(L // CIW):
                njt = (ci + 1) * CIW // 128
                offs = [128 * jt - CIW * ci for jt in range(njt)]
                # per stream (even/odd head), per column half: one psum bank
                yps = [
                    [
                        ps_y.tile([128, 512], f32, tag=f"yp{e}{h}",
                                  name=f"yp{e}{h}")
                        for h in range(2)
                    ]
                    for e in range(2)
                ]
                pts = [[None] * njt for _ in range(2)]

                def emit_qk(e, jt):
                    hh = 2 * hp + e
                    off = offs[jt]
                    o = 64 * e
                    sp = ps_s.tile([128, CIW], f32, tag=f"sp{e}", name=f"sp{e}")
                    for lo in range(0, CIW, 512):
                        if off >= lo + 512:
                            continue
                        nc.tensor.matmul(
                            sp[:, lo:lo + 512],
                            kz[hp][o:o + 64, jt * 128:(jt + 1) * 128],
                            q_t[hp][o:o + 64, ci * CIW + lo: ci * CIW + lo + 512],
                            start=True,
                            stop=True,
                        )
                    pt = ppool.tile([128, CIW], f32r, tag="pt", name="ptt")
                    a = max(off, 0)
                    if off > 0 and off % 512:
                        nc.gpsimd.memset(
                            pt[:, (off // 512) * 512: off].bitcast(f32), 0.0
                        )
                    nc.scalar.activation(
                        pt[:, a:CIW],
                        sp[:, a:CIW],
                        mybir.ActivationFunctionType.Exp,
                        scale=float(1.0 / np.sqrt(HD)),
                    )
                    if off >= 0:
                        nc.vector.tensor_mul(
                            pt[:, off:off + 128], pt[:, off:off + 128], tri[:]
                        )
                    pts[e][jt] = pt

                def emit_pv_group(e, g0, g1):
                    hh = 2 * hp + e
                    for lo in range(0, CIW, 512):
                        last = min(njt - 1, (CIW * ci + lo + 511) // 128)
                        for jt in range(g0, g1):
                            off = offs[jt]
                            if off >= lo + 512:
                                continue
                            nc.tensor.matmul(
                                yps[e][lo // 512][:],
                                vones[jt][:, hh * VW: hh * VW + 128],
                                pts[e][jt][:, lo:lo + 512],
                                start=(jt == 0),
                                stop=(jt == last),
                            )

                for g0 in range(0, njt, GRP):
                    g1 = min(g0 + GRP, njt)
                    for jt in range(g0, g1):
                        emit_qk(0, jt)
                        emit_qk(1, jt)
                    if g0 > 0:
                        emit_pv_group(0, g0 - GRP, g0)
                        emit_pv_group(1, g0 - GRP, g0)
                g0 = (njt - 1) // GRP * GRP
                emit_pv_group(0, g0, njt)
                emit_pv_group(1, g0, njt)

                # yT = y' / rowsum; rowsum lives in psum row 64
                for e in range(2):
                    hh = 2 * hp + e
                    for h in range(2):
                        yp = yps[e][h]
                        cols = slice(ci * CIW + 512 * h, ci * CIW + 512 * (h + 1))
                        rrec = rpool.tile([1, 512], f32, tag="rrec", name="rrect")
                        nc.vector.reciprocal(rrec[:], yp[64:65, :])
                        rrecb = rpool.tile([64, 512], f32, tag="rrecb",
                                           name="rrecbt")
                        nc.gpsimd.partition_broadcast(rrecb[:], rrec[:])
                        nc.vector.tensor_mul(
                            yT[hp][64 * e: 64 * e + 64, cols],
                            yp[0:64, :],
                            rrecb[:],
                        )


def _phase3(nc, tc, yT, wproj, out):
    """Output projection (partial: host adds the two head groups)."""
    with (
        tc.tile_pool(name="wp", bufs=1) as wppool,
        tc.tile_pool(name="ob", bufs=3) as opool,
        tc.tile_pool(name="ps3", bufs=4, space="PSUM") as ps3,
    ):
        wp_sb = []
        for ft in range(FT):
            wp_t = wppool.tile([128, D], f32r, tag=f"wp{ft}", name=f"wp{ft}")
            nc.sync.dma_start(
                out=wp_t[:], in_=wproj[ft * 128:(ft + 1) * 128, :].bitcast(f32r)
            )
            wp_sb.append(wp_t)
        for it in range(MT):
            o_t = opool.tile([128, D], f32, tag="ot", name="ott")
            for nc_ in range(D // 512):
                ps = ps3.tile([128, 512], f32, tag="ps3", name="ps3t")
                for ft in range(FT):
                    nc.tensor.matmul(
                        ps[:],
                        yT[ft][:, it * 128:(it + 1) * 128],
                        wp_sb[ft][:, nc_ * 512:(nc_ + 1) * 512],
                        start=(ft == 0),
                        stop=(ft == FT - 1),
                    )
                nc.vector.tensor_copy(o_t[:, nc_ * 512:(nc_ + 1) * 512], ps[:])
            nc.sync.dma_start(out=out[it * 128:(it + 1) * 128, :], in_=o_t[:])


def _emit(nc, tc, xT, wqkv, wproj, trimask, ones8, out):
    with tc.tile_pool(name="persist", bufs=1) as persist:
        trimask_sb = persist.tile([128, 128], f32, tag="trif", name="trif")
        nc.sync.dma_start(out=trimask_sb[:], in_=trimask[:, :])
        qkdt = bf16 if QK_MODE == "bf16z" else f32r
        # q: 2 heads per tile (rows 0-63 even head, 64-127 odd head)
        q_t = [
            persist.tile([128, L], qkdt, tag=f"q{i}", name=f"q{i}")
            for i in range(4)
        ]
        # k: bf16z mode: one zero-padded tile per head (even heads in rows
        # 0-63, odd heads rows 64-127, other half zero). f32r64 mode: 2
        # heads per tile like q.
        nkz = HPC if QK_MODE == "bf16z" else 4
        kz = [
            persist.tile([128, L], qkdt, tag=f"kz{hh}", name=f"kz{hh}")
            for hh in range(nkz)
        ]
        # v natural layout + ones column per head, flat [128, 8*65(+pad)]
        # (PV reads a 128-wide window starting at each head's slot)
        vones = [
            persist.tile([128, (HPC - 1) * VW + 128], f32r, tag=f"vo{mt}",
                         name=f"vo{mt}")
            for mt in range(MT)
        ]
        tri = persist.tile([128, 128], f32r, tag="tri")
        nc.vector.tensor_copy(tri[:], trimask_sb[:])
        for mt in range(MT):
            vview = vones[mt][:, 0:HPC * VW].rearrange("p (h c) -> p h c", c=VW)
            nc.sync.dma_start(
                out=vview[:, :, HD], in_=ones8[:, :].bitcast(f32r)
            )

        with nc.named_scope("phase1"):
            _phase1(nc, tc, xT, wqkv, q_t, kz, vones)
        with tc.tile_pool(name="ph23", bufs=1) as ph23:
            yT = [
                ph23.tile([128, L], f32r, tag=f"yT{ft}", name=f"yT{ft}")
                for ft in range(FT)
            ]
            with nc.named_scope("phase2"):
                _phase2(nc, tc, q_t, kz, vones, tri, yT)
            with nc.named_scope("phase3"):
                _phase3(nc, tc, yT, wproj, out)


def build():
    nc = bacc.Bacc(
        "TRN2", target_bir_lowering=False, debug=False, num_devices=N_CORES
    )
    xT = nc.dram_tensor("xT", [D, L], f32, kind="ExternalInput").ap()
    wqkv = nc.dram_tensor("wqkv", [D, 3 * DG], f32, kind="ExternalInput").ap()
    wproj = nc.dram_tensor("wproj", [DG, D], f32, kind="ExternalInput").ap()
    trimask = nc.dram_tensor("trimask", [128, 128], f32, kind="ExternalInput").ap()
    ones8 = nc.dram_tensor("ones8", [128, HPC], f32, kind="ExternalInput").ap()
    out = nc.dram_tensor("out", [L, D], f32, kind="ExternalOutput").ap()
    with tile.TileContext(nc) as tc:
        _emit(nc, tc, xT, wqkv, wproj, trimask, ones8, out)
    nc.compile()
    return nc


def shard_inputs(x, Wqkv, Wproj):
    tri = np.triu(np.ones((128, 128), dtype=np.float32))
    in_maps = []
    for c in range(N_CORES):
        b, g = c // 2, c % 2
        wqkv_c = np.concatenate(
            [
                Wqkv[:, DG * g:DG * (g + 1)],
                Wqkv[:, D + DG * g:D + DG * (g + 1)],
                Wqkv[:, 2 * D + DG * g:2 * D + DG * (g + 1)],
            ],
            axis=1,
        )
        in_maps.append(
            {
                "xT": np.ascontiguousarray(x[b].T),
                "wqkv": np.ascontiguousarray(wqkv_c),
                "wproj": np.ascontiguousarray(Wproj[DG * g:DG * (g + 1), :]),
                "trimask": tri,
                "ones8": np.ones((128, HPC), dtype=np.float32),
            }
        )
    return in_maps


_NC_CACHE = {}


def get_nc():
    if "nc" not in _NC_CACHE:
        _NC_CACHE["nc"] = build()
    return _NC_CACHE["nc"]


def run_sharded(in_maps, **kwargs):
    return run_bass_kernel_spmd(
        get_nc(), in_maps, core_ids=list(range(N_CORES)), **kwargs
    )


def kernel(x, Wqkv, Wproj, attn_mask, key_padding_mask):
    # attn_mask is causal and key_padding_mask is all-False for this
    # problem; both are hardcoded into the device program.
    x = np.asarray(x, dtype=np.float32)
    in_maps = shard_inputs(
        x, np.asarray(Wqkv, dtype=np.float32), np.asarray(Wproj, dtype=np.float32)
    )
    res = run_sharded(in_maps)
    out = np.empty((B, L, D), dtype=np.float32)
    for b in range(B):
        out[b] = res.results[2 * b]["out"] + res.results[2 * b + 1]["out"]
    return out


# revision 15
# speedup vs baseline: 1.0801x; 1.0801x over previous
"""Causal self-attention on 8 Trainium2 cores.

Sharding: core c handles batch b = c // 2 and head group g = c % 2
(8 of 16 heads). Wqkv is split column-wise by head (tensor parallel),
Wproj row-wise; the host sums the two partial outputs per batch
(the all-reduce step).

Self-contained: hardcodes B=4, L=2048, D=1024, H=16.
"""

import numpy as np

import concourse.bass as bass  # noqa: F401  (bass types via bacc/tile)
import concourse.mybir as mybir
import concourse.tile as tile
from concourse import bacc
from concourse.bass_utils import run_bass_kernel_spmd

B, L, D, H, HD = 4, 2048, 1024, 16, 64
N_CORES = 8
HPC = 8            # heads per core
DG = HPC * HD      # 512: feature columns per head group
KT = D // 128      # 8 contraction tiles for the input projections
CIW = 1024         # query-chunk width in the attention phase
PVLAG = 2          # j-tiles of lag between the QK/exp stream and PV

f32 = mybir.dt.float32
f32r = mybir.dt.float32r


def _emit(nc, tc, xT, wqkv, wproj, trimask, ones8, out):
    FT = DG // 128  # 4 partition tiles of yT / wproj contraction
    with tc.tile_pool(name="persist", bufs=1) as persist:
        # q,k in transposed layout: rows = feature (q: 0-511, k: 512-1023),
        # cols = token. 8 partition tiles of [128, L].
        qkT = [persist.tile([128, L], f32r, tag=f"qkT{nt}", name=f"qkT{nt}") for nt in range(8)]
        # v in natural layout [token, head, hd+1]; last col = 1.0 so the PV
        # matmul also produces the softmax denominator (row 64 of its psum).
        vones = [
            persist.tile([128, HPC, HD + 1], f32r, tag=f"vo{mt}", name=f"vo{mt}")
            for mt in range(L // 128)
        ]
        tri = persist.tile([128, 128], f32r, tag="tri")
        nc.sync.dma_start(out=tri[:], in_=trimask[:, :].bitcast(f32r))

        # ---- phase 1: input projections ----
        with (
            tc.tile_pool(name="wq", bufs=1) as wpool,
            tc.tile_pool(name="xt", bufs=2) as xtpool,
            tc.tile_pool(name="ps1", bufs=6, space="PSUM") as ps1,
        ):
            w_sb = []
            for kt in range(KT):
                w_t = wpool.tile([128, 3 * DG], f32r, tag=f"w{kt}", name=f"w{kt}")
                nc.sync.dma_start(
                    out=w_t[:], in_=wqkv[kt * 128:(kt + 1) * 128, :].bitcast(f32r)
                )
                w_sb.append(w_t)
            for mc in range(L // 512):
                xts = []
                for kt in range(KT):
                    xt_t = xtpool.tile([128, 512], f32r, tag=f"xt{kt}", name=f"xt{kt}")
                    nc.sync.dma_start(
                        out=xt_t[:],
                        in_=xT[
                            kt * 128:(kt + 1) * 128, mc * 512:(mc + 1) * 512
                        ].bitcast(f32r),
                    )
                    xts.append(xt_t)
                # q,k → transposed layout
                for nt in range(8):
                    ps = ps1.tile([128, 512], f32, tag="ps1", name="ps1t")
                    for kt in range(KT):
                        nc.tensor.matmul(
                            ps[:],
                            w_sb[kt][:, nt * 128:(nt + 1) * 128],
                            xts[kt][:],
                            start=(kt == 0),
                            stop=(kt == KT - 1),
                        )
                    nc.vector.tensor_copy(
                        qkT[nt][:, mc * 512:(mc + 1) * 512], ps[:]
                    )
                # v → natural layout, interleaved with the ones column
                for mi in range(4):
                    mt = mc * 4 + mi
                    ps = ps1.tile([128, 512], f32, tag="ps1", name="ps1t")
                    for kt in range(KT):
                        nc.tensor.matmul(
                            ps[:],
                            xts[kt][:, mi * 128:(mi + 1) * 128],
                            w_sb[kt][:, 2 * DG:3 * DG],
                            start=(kt == 0),
                            stop=(kt == KT - 1),
                        )
                    nc.vector.tensor_copy(
                        vones[mt][:, :, 0:HD],
                        ps[:].rearrange("p (h d) -> p h d", d=HD),
                    )
                    nc.sync.dma_start(
                        out=vones[mt][:, :, HD], in_=ones8[:, :].bitcast(f32r)
                    )

        # ---- phases 2+3 ----
        with tc.tile_pool(name="ph23", bufs=1) as ph23:
            yT = [
                ph23.tile([128, L], f32r, tag=f"yT{ft}", name=f"yT{ft}")
                for ft in range(FT)
            ]
            _phase2(nc, tc, qkT, vones, tri, yT)
            _phase3(nc, tc, yT, wproj, out)


def _phase2(nc, tc, qkT, vones, tri, yT):
        # attention (S^T layout: rows=key j, cols=query i)
        with (
            tc.tile_pool(name="pp", bufs=6) as ppool,
            tc.tile_pool(name="rr", bufs=2) as rpool,
            tc.tile_pool(name="pss", bufs=2, space="PSUM") as ps_s,
            tc.tile_pool(name="psy", bufs=2, space="PSUM") as ps_y,
        ):
            for hh in range(HPC):
                q_t, q_off = qkT[hh // 2], 64 * (hh % 2)
                k_t, k_off = qkT[4 + hh // 2], 64 * (hh % 2)
                for ci in range(L // CIW):
                    njt = (ci + 1) * CIW // 128
                    yp = ps_y.tile([65, CIW], f32, tag="yp", name="ypt")
                    ptiles = [None] * njt
                    offs = [128 * jt - CIW * ci for jt in range(njt)]

                    def emit_qk(jt):
                        off = offs[jt]
                        sp = ps_s.tile([128, CIW], f32, tag="sp", name="spt")
                        for lo in range(0, CIW, 512):
                            if off >= lo + 512:
                                continue  # fully masked column range
                            nc.tensor.matmul(
                                sp[:, lo:lo + 512],
                                k_t[k_off:k_off + 64, jt * 128:(jt + 1) * 128],
                                q_t[
                                    q_off:q_off + 64,
                                    ci * CIW + lo: ci * CIW + lo + 512,
                                ],
                                start=True,
                                stop=True,
                            )
                        pt = ppool.tile([128, CIW], f32r, tag="pt", name="ptt")
                        a = max(off, 0)
                        if off > 0 and off % 512:
                            nc.gpsimd.memset(
                                pt[:, (off // 512) * 512: off].bitcast(f32), 0.0
                            )
                        nc.scalar.activation(
                            pt[:, a:CIW],
                            sp[:, a:CIW],
                            mybir.ActivationFunctionType.Exp,
                            scale=float(1.0 / np.sqrt(HD)),
                        )
                        if off >= 0:
                            nc.vector.tensor_mul(
                                pt[:, off:off + 128], pt[:, off:off + 128], tri[:]
                            )
                        ptiles[jt] = pt

                    def emit_pv(jt):
                        off = offs[jt]
                        for lo in range(0, CIW, 512):
                            if off >= lo + 512:
                                continue
                            # last j-tile contributing to this column range
                            last = min(njt - 1, (CIW * ci + lo + 511) // 128)
                            nc.tensor.matmul(
                                yp[:, lo:lo + 512],
                                vones[jt][:, hh, :],
                                ptiles[jt][:, lo:lo + 512],
                                start=(jt == 0),
                                stop=(jt == last),
                            )

                    for jt in range(njt + PVLAG):
                        if jt < njt:
                            emit_qk(jt)
                        if jt - PVLAG >= 0:
                            emit_pv(jt - PVLAG)

                    # normalize: yT = y' / rowsum (rowsum lives in row 64).
                    # 1/r computed as exp(-ln r) on the scalar engine: the
                    # DVE reciprocal is an 8-pass iterative op (6.5us/call)
                    rln = rpool.tile([1, CIW], f32, tag="rln", name="rlnt")
                    nc.scalar.activation(
                        rln[:], yp[64:65, :], mybir.ActivationFunctionType.Ln
                    )
                    rrec = rpool.tile([1, CIW], f32, tag="rrec", name="rrect")
                    nc.scalar.activation(
                        rrec[:], rln[:], mybir.ActivationFunctionType.Exp,
                        scale=-1.0,
                    )
                    rrecb = rpool.tile([64, CIW], f32, tag="rrecb", name="rrecbt")
                    nc.gpsimd.partition_broadcast(rrecb[:], rrec[:])
                    nc.vector.tensor_mul(
                        yT[hh // 2][
                            64 * (hh % 2): 64 * (hh % 2) + 64,
                            ci * CIW:(ci + 1) * CIW,
                        ],
                        yp[0:64, :],
                        rrecb[:],
                    )


def _phase3(nc, tc, yT, wproj, out):
        # output projection (partial: host adds the two head groups)
        FT = DG // 128
        with (
            tc.tile_pool(name="wp", bufs=1) as wppool,
            tc.tile_pool(name="ob", bufs=3) as opool,
            tc.tile_pool(name="ps3", bufs=4, space="PSUM") as ps3,
        ):
            wp_sb = []
            for ft in range(FT):
                wp_t = wppool.tile([128, D], f32r, tag=f"wp{ft}", name=f"wp{ft}")
                nc.sync.dma_start(
                    out=wp_t[:], in_=wproj[ft * 128:(ft + 1) * 128, :].bitcast(f32r)
                )
                wp_sb.append(wp_t)
            for it in range(L // 128):
                o_t = opool.tile([128, D], f32, tag="ot", name="ott")
                for nc_ in range(D // 512):
                    ps = ps3.tile([128, 512], f32, tag="ps3", name="ps3t")
                    for ft in range(FT):
                        nc.tensor.matmul(
                            ps[:],
                            yT[ft][:, it * 128:(it + 1) * 128],
                            wp_sb[ft][:, nc_ * 512:(nc_ + 1) * 512],
                            start=(ft == 0),
                            stop=(ft == FT - 1),
                        )
                    nc.vector.tensor_copy(o_t[:, nc_ * 512:(nc_ + 1) * 512], ps[:])
                nc.sync.dma_start(
                    out=out[it * 128:(it + 1) * 128, :], in_=o_t[:]
                )


def build():
    nc = bacc.Bacc(
        "TRN2", target_bir_lowering=False, debug=False, num_devices=N_CORES
    )
    xT = nc.dram_tensor("xT", [D, L], f32, kind="ExternalInput").ap()
    wqkv = nc.dram_tensor("wqkv", [D, 3 * DG], f32, kind="ExternalInput").ap()
    wproj = nc.dram_tensor("wproj", [DG, D], f32, kind="ExternalInput").ap()
    trimask = nc.dram_tensor("trimask", [128, 128], f32, kind="ExternalInput").ap()
    ones8 = nc.dram_tensor("ones8", [128, HPC], f32, kind="ExternalInput").ap()
    out = nc.dram_tensor("out", [L, D], f32, kind="ExternalOutput").ap()
    with tile.TileContext(nc) as tc:
        _emit(nc, tc, xT, wqkv, wproj, trimask, ones8, out)
    nc.compile()
    return nc


def shard_inputs(x, Wqkv, Wproj):
    tri = np.triu(np.ones((128, 128), dtype=np.float32))
    in_maps = []
    for c in range(N_CORES):
        b, g = c // 2, c % 2
        wqkv_c = np.concatenate(
            [
                Wqkv[:, DG * g:DG * (g + 1)],
                Wqkv[:, D + DG * g:D + DG * (g + 1)],
                Wqkv[:, 2 * D + DG * g:2 * D + DG * (g + 1)],
            ],
            axis=1,
        )
        in_maps.append(
            {
                "xT": np.ascontiguousarray(x[b].T),
                "wqkv": np.ascontiguousarray(wqkv_c),
                "wproj": np.ascontiguousarray(Wproj[DG * g:DG * (g + 1), :]),
                "trimask": tri,
                "ones8": np.ones((128, HPC), dtype=np.float32),
            }
        )
    return in_maps


_NC_CACHE = {}


def get_nc():
    if "nc" not in _NC_CACHE:
        _NC_CACHE["nc"] = build()
    return _NC_CACHE["nc"]


def run_sharded(in_maps, **kwargs):
    return run_bass_kernel_spmd(
        get_nc(), in_maps, core_ids=list(range(N_CORES)), **kwargs
    )


def kernel(x, Wqkv, Wproj, attn_mask, key_padding_mask):
    # attn_mask is causal and key_padding_mask is all-False for this
    # problem; both are hardcoded into the device program.
    x = np.asarray(x, dtype=np.float32)
    in_maps = shard_inputs(
        x, np.asarray(Wqkv, dtype=np.float32), np.asarray(Wproj, dtype=np.float32)
    )
    res = run_sharded(in_maps)
    out = np.empty((B, L, D), dtype=np.float32)
    for b in range(B):
        out[b] = res.results[2 * b]["out"] + res.results[2 * b + 1]["out"]
    return out


# revision 16
# speedup vs baseline: 1.0968x; 1.0154x over previous
"""Causal self-attention on 8 Trainium2 cores.

Sharding: core c handles batch b = c // 2 and head group g = c % 2
(8 of 16 heads). Wqkv is split column-wise by head (tensor parallel),
Wproj row-wise; the host sums the two partial outputs per batch
(the all-reduce step).

Self-contained: hardcodes B=4, L=2048, D=1024, H=16.
"""

import numpy as np

import concourse.bass as bass  # noqa: F401  (bass types via bacc/tile)
import concourse.mybir as mybir
import concourse.tile as tile
from concourse import bacc
from concourse.bass_utils import run_bass_kernel_spmd

B, L, D, H, HD = 4, 2048, 1024, 16, 64
N_CORES = 8
HPC = 8            # heads per core
DG = HPC * HD      # 512: feature columns per head group
KT = D // 128      # 8 contraction tiles for the input projections
CIW = 1024         # query-chunk width in the attention phase
PVLAG = 4          # j-tiles of lag between the QK/exp stream and PV

f32 = mybir.dt.float32
f32r = mybir.dt.float32r


def _emit(nc, tc, xT, wqkv, wproj, trimask, ones8, out):
    FT = DG // 128  # 4 partition tiles of yT / wproj contraction
    with tc.tile_pool(name="persist", bufs=1) as persist:
        # q,k in transposed layout: rows = feature (q: 0-511, k: 512-1023),
        # cols = token. 8 partition tiles of [128, L].
        qkT = [persist.tile([128, L], f32r, tag=f"qkT{nt}", name=f"qkT{nt}") for nt in range(8)]
        # v in natural layout [token, head, hd+1]; last col = 1.0 so the PV
        # matmul also produces the softmax denominator (row 64 of its psum).
        vones = [
            persist.tile([128, HPC, HD + 1], f32r, tag=f"vo{mt}", name=f"vo{mt}")
            for mt in range(L // 128)
        ]
        tri = persist.tile([128, 128], f32r, tag="tri")
        nc.sync.dma_start(out=tri[:], in_=trimask[:, :].bitcast(f32r))

        # ---- phase 1: input projections ----
        with (
            tc.tile_pool(name="wq", bufs=1) as wpool,
            tc.tile_pool(name="xt", bufs=2) as xtpool,
            tc.tile_pool(name="ps1", bufs=6, space="PSUM") as ps1,
        ):
            w_sb = []
            for kt in range(KT):
                w_t = wpool.tile([128, 3 * DG], f32r, tag=f"w{kt}", name=f"w{kt}")
                nc.sync.dma_start(
                    out=w_t[:], in_=wqkv[kt * 128:(kt + 1) * 128, :].bitcast(f32r)
                )
                w_sb.append(w_t)
            for mc in range(L // 512):
                xts = []
                for kt in range(KT):
                    xt_t = xtpool.tile([128, 512], f32r, tag=f"xt{kt}", name=f"xt{kt}")
                    nc.sync.dma_start(
                        out=xt_t[:],
                        in_=xT[
                            kt * 128:(kt + 1) * 128, mc * 512:(mc + 1) * 512
                        ].bitcast(f32r),
                    )
                    xts.append(xt_t)
                # q,k → transposed layout
                for nt in range(8):
                    ps = ps1.tile([128, 512], f32, tag="ps1", name="ps1t")
                    for kt in range(KT):
                        nc.tensor.matmul(
                            ps[:],
                            w_sb[kt][:, nt * 128:(nt + 1) * 128],
                            xts[kt][:],
                            start=(kt == 0),
                            stop=(kt == KT - 1),
                        )
                    nc.vector.tensor_copy(
                        qkT[nt][:, mc * 512:(mc + 1) * 512], ps[:]
                    )
                # v → natural layout, interleaved with the ones column
                for mi in range(4):
                    mt = mc * 4 + mi
                    ps = ps1.tile([128, 512], f32, tag="ps1", name="ps1t")
                    for kt in range(KT):
                        nc.tensor.matmul(
                            ps[:],
                            xts[kt][:, mi * 128:(mi + 1) * 128],
                            w_sb[kt][:, 2 * DG:3 * DG],
                            start=(kt == 0),
                            stop=(kt == KT - 1),
                        )
                    nc.vector.tensor_copy(
                        vones[mt][:, :, 0:HD],
                        ps[:].rearrange("p (h d) -> p h d", d=HD),
                    )
                    nc.sync.dma_start(
                        out=vones[mt][:, :, HD], in_=ones8[:, :].bitcast(f32r)
                    )

        # ---- phases 2+3 ----
        with tc.tile_pool(name="ph23", bufs=1) as ph23:
            yT = [
                ph23.tile([128, L], f32r, tag=f"yT{ft}", name=f"yT{ft}")
                for ft in range(FT)
            ]
            _phase2(nc, tc, qkT, vones, tri, yT)
            _phase3(nc, tc, yT, wproj, out)


def _phase2(nc, tc, qkT, vones, tri, yT):
        # attention (S^T layout: rows=key j, cols=query i)
        with (
            tc.tile_pool(name="pp", bufs=8) as ppool,
            tc.tile_pool(name="rr", bufs=2) as rpool,
            tc.tile_pool(name="pss", bufs=2, space="PSUM") as ps_s,
            tc.tile_pool(name="psy", bufs=2, space="PSUM") as ps_y,
        ):
            for hh in range(HPC):
                q_t, q_off = qkT[hh // 2], 64 * (hh % 2)
                k_t, k_off = qkT[4 + hh // 2], 64 * (hh % 2)
                for ci in range(L // CIW):
                    njt = (ci + 1) * CIW // 128
                    yp = ps_y.tile([65, CIW], f32, tag="yp", name="ypt")
                    ptiles = [None] * njt
                    offs = [128 * jt - CIW * ci for jt in range(njt)]

                    def emit_qk(jt):
                        off = offs[jt]
                        sp = ps_s.tile([128, CIW], f32, tag="sp", name="spt")
                        for lo in range(0, CIW, 512):
                            if off >= lo + 512:
                                continue  # fully masked column range
                            nc.tensor.matmul(
                                sp[:, lo:lo + 512],
                                k_t[k_off:k_off + 64, jt * 128:(jt + 1) * 128],
                                q_t[
                                    q_off:q_off + 64,
                                    ci * CIW + lo: ci * CIW + lo + 512,
                                ],
                                start=True,
                                stop=True,
                            )
                        pt = ppool.tile([128, CIW], f32r, tag="pt", name="ptt")
                        a = max(off, 0)
                        if off > 0 and off % 512:
                            nc.gpsimd.memset(
                                pt[:, (off // 512) * 512: off].bitcast(f32), 0.0
                            )
                        nc.scalar.activation(
                            pt[:, a:CIW],
                            sp[:, a:CIW],
                            mybir.ActivationFunctionType.Exp,
                            scale=float(1.0 / np.sqrt(HD)),
                        )
                        if off >= 0:
                            nc.vector.tensor_mul(
                                pt[:, off:off + 128], pt[:, off:off + 128], tri[:]
                            )
                        ptiles[jt] = pt

                    def emit_pv(jt):
                        off = offs[jt]
                        for lo in range(0, CIW, 512):
                            if off >= lo + 512:
                                continue
                            # last j-tile contributing to this column range
                            last = min(njt - 1, (CIW * ci + lo + 511) // 128)
                            nc.tensor.matmul(
                                yp[:, lo:lo + 512],
                                vones[jt][:, hh, :],
                                ptiles[jt][:, lo:lo + 512],
                                start=(jt == 0),
                                stop=(jt == last),
                            )

                    for jt in range(njt + PVLAG):
                        if jt < njt:
                            emit_qk(jt)
                        if jt - PVLAG >= 0:
                            emit_pv(jt - PVLAG)

                    # normalize: yT = y' / rowsum (rowsum lives in row 64).
                    # 1/r computed as exp(-ln r) on the scalar engine: the
                    # DVE reciprocal is an 8-pass iterative op (6.5us/call)
                    rln = rpool.tile([1, CIW], f32, tag="rln", name="rlnt")
                    nc.scalar.activation(
                        rln[:], yp[64:65, :], mybir.ActivationFunctionType.Ln
                    )
                    rrec = rpool.tile([1, CIW], f32, tag="rrec", name="rrect")
                    nc.scalar.activation(
                        rrec[:], rln[:], mybir.ActivationFunctionType.Exp,
                        scale=-1.0,
                    )
                    rrecb = rpool.tile([64, CIW], f32, tag="rrecb", name="rrecbt")
                    nc.gpsimd.partition_broadcast(rrecb[:], rrec[:])
                    nc.vector.tensor_mul(
                        yT[hh // 2][
                            64 * (hh % 2): 64 * (hh % 2) + 64,
                            ci * CIW:(ci + 1) * CIW,
                        ],
                        yp[0:64, :],
                        rrecb[:],
                    )


def _phase3(nc, tc, yT, wproj, out):
        # output projection (partial: host adds the two head groups)
        FT = DG // 128
        with (
            tc.tile_pool(name="wp", bufs=1) as wppool,
            tc.tile_pool(name="ob", bufs=3) as opool,
            tc.tile_pool(name="ps3", bufs=4, space="PSUM") as ps3,
        ):
            wp_sb = []
            for ft in range(FT):
                wp_t = wppool.tile([128, D], f32r, tag=f"wp{ft}", name=f"wp{ft}")
                nc.sync.dma_start(
                    out=wp_t[:], in_=wproj[ft * 128:(ft + 1) * 128, :].bitcast(f32r)
                )
                wp_sb.append(wp_t)
            for it in range(L // 128):
                o_t = opool.tile([128, D], f32, tag="ot", name="ott")
                for nc_ in range(D // 512):
                    ps = ps3.tile([128, 512], f32, tag="ps3", name="ps3t")
                    for ft in range(FT):
                        nc.tensor.matmul(
                            ps[:],
                            yT[ft][:, it * 128:(it + 1) * 128],
                            wp_sb[ft][:, nc_ * 512:(nc_ + 1) * 512],
                            start=(ft == 0),
                            stop=(ft == FT - 1),
                        )
                    nc.vector.tensor_copy(o_t[:, nc_ * 512:(nc_ + 1) * 512], ps[:])
                nc.sync.dma_start(
                    out=out[it * 128:(it + 1) * 128, :], in_=o_t[:]
                )


def build():
    nc = bacc.Bacc(
        "TRN2", target_bir_lowering=False, debug=False, num_devices=N_CORES
    )
    xT = nc.dram_tensor("xT", [D, L], f32, kind="ExternalInput").ap()
    wqkv = nc.dram_tensor("wqkv", [D, 3 * DG], f32, kind="ExternalInput").ap()
    wproj = nc.dram_tensor("wproj", [DG, D], f32, kind="ExternalInput").ap()
    trimask = nc.dram_tensor("trimask", [128, 128], f32, kind="ExternalInput").ap()
    ones8 = nc.dram_tensor("ones8", [128, HPC], f32, kind="ExternalInput").ap()
    out = nc.dram_tensor("out", [L, D], f32, kind="ExternalOutput").ap()
    with tile.TileContext(nc) as tc:
        _emit(nc, tc, xT, wqkv, wproj, trimask, ones8, out)
    nc.compile()
    return nc


def shard_inputs(x, Wqkv, Wproj):
    tri = np.triu(np.ones((128, 128), dtype=np.float32))
    in_maps = []
    for c in range(N_CORES):
        b, g = c // 2, c % 2
        wqkv_c = np.concatenate(
            [
                Wqkv[:, DG * g:DG * (g + 1)],
                Wqkv[:, D + DG * g:D + DG * (g + 1)],
                Wqkv[:, 2 * D + DG * g:2 * D + DG * (g + 1)],
            ],
            axis=1,
        )
        in_maps.append(
            {
                "xT": np.ascontiguousarray(x[b].T),
                "wqkv": np.ascontiguousarray(wqkv_c),
                "wproj": np.ascontiguousarray(Wproj[DG * g:DG * (g + 1), :]),
                "trimask": tri,
                "ones8": np.ones((128, HPC), dtype=np.float32),
            }
        )
    return in_maps


_NC_CACHE = {}


def get_nc():
    if "nc" not in _NC_CACHE:
        _NC_CACHE["nc"] = build()
    return _NC_CACHE["nc"]


def run_sharded(in_maps, **kwargs):
    return run_bass_kernel_spmd(
        get_nc(), in_maps, core_ids=list(range(N_CORES)), **kwargs
    )


def kernel(x, Wqkv, Wproj, attn_mask, key_padding_mask):
    # attn_mask is causal and key_padding_mask is all-False for this
    # problem; both are hardcoded into the device program.
    x = np.asarray(x, dtype=np.float32)
    in_maps = shard_inputs(
        x, np.asarray(Wqkv, dtype=np.float32), np.asarray(Wproj, dtype=np.float32)
    )
    res = run_sharded(in_maps)
    out = np.empty((B, L, D), dtype=np.float32)
    for b in range(B):
        out[b] = res.results[2 * b]["out"] + res.results[2 * b + 1]["out"]
    return out
